# revision 31
# baseline (speedup 1.0000x reference)
"""Normalized-adjacency kernel (EstimateAdj.normalize, symmetric=False) for TRN2.

out = mx * r_inv[:, None] * r_inv[None, :]   where mx = adj + I,
r_inv = rowsum(mx) ** -0.5.

Strategy (8 NeuronCores, row-sharded, raw Bass with explicit semaphores):
  - host: add 1.0 to the diagonal (O(n)), split rows into 8 shards
  - device, per core (shard [1024, 8192], half-tiles [128 x 4096], 16 items):
      pass 1: item 15 is loaded FIRST as four single-block quarters into a
              dedicated f32 tile: the first rowsum activation starts ~4 us
              into the kernel (vs ~19 us for a full half-tile), which
              un-jams the whole slot-gated load pipeline.  Items 0..14
              stream through 3 f32 slots, loads alternating Pool/SP rings.
              Each chunk is consumed by ONE scalar-engine activation (Copy
              with accum_out): the accumulator is the rowsum partial and
              the Copy output writes a bf16 replica into a persistent SBUF
              cache (the dedicated tile copies in place, staying f32).
              The 32 MiB shard is cached on-chip, NEVER reloaded from HBM.
      Rowsum folds + sqrt stay ON the scalar engine (accumulator
      writebacks land asynchronously; only self-waits drain them --
      cross-engine readers of accum targets are a race).
      r_inv = 1/sqrt(rowsum), transposed via PE, bf16 via DVE reciprocal.
      The AllGather is SPLIT IN TWO at the tile-4 boundary.  The first
      gather's input is ready at ~55% of the load phase, so its ~12 us
      setup and the ~25-35 us cross-core start skew (the runtime
      staggers core launches) largely complete while loads stream; the
      second rides the warm CC pipeline (~2 us setup) and releases
      before the store stream reaches the columns it gates.
      cc_in writes + colscale broadcasts ride the Scalar engine's
      otherwise-empty DMA queue; collectives trigger on gpsimd (NRT
      requires straight-line gpsimd ordering).
      pass 2: per item and phase x (columns j with (j mod 1024) in
      [512x, 512x+512) are phase-x columns = rows of tile-group x), DVE
      scalar_tensor_tensor over block-strided views:
              slot[:, :, lo:hi] = (cache * r_inv_row) * colscale_x
      then a block-strided store (2 KB descriptors), alternating SP/Pool
      rings.  Stores start the moment the core's own loads drain; the
      collective latency hides under the load/store streams.
  - bf16 rounding of mx and colscale adds ~8e-3 relative error -- inside
    the 2e-2 gate; rowsums/r_inv row scalars stay f32.
  - host: concatenate the 8 output shards

HBM traffic per core: 32 MiB load + 32 MiB store + 2 MiB colscale (vs the
two-pass baseline's 92 MiB) -> DMA-bound floor ~190 us at 360 GB/s.
"""

from contextlib import ExitStack

import numpy as np

import concourse.bass as bass
import concourse.mybir as mybir
from concourse.bass_utils import run_bass_kernel_spmd

N = 8192
NCORES = 8
SHARD = N // NCORES  # 1024
P = 128
T = SHARD // P  # 8 tiles per core
H = 2  # column halves per tile
W = N // H  # 4096
B = 4  # 1024-col blocks per half
V = W // B  # 1024

F32 = mybir.dt.float32
BF16 = mybir.dt.bfloat16
NSLOTS = 3  # f32 streaming slots (pass-1 loads / pass-2 outputs)
GROUPS = [(0, 4), (4, 8)]  # tile groups -> gather phases
NG = len(GROUPS)
NQ = 4  # dedicated-tile load quarters (single blocks)


def build_kernel():
    items = [(t, h) for t in range(T) for h in range(H)]
    ni = len(items)
    last = ni - 1  # item 15: dedicated f32 tile, scaled in place
    # load positions: 4 ded quarters first, then items 0..14
    lorder = [("q", q) for q in range(NQ)] + [("i", i) for i in range(ni - 1)]
    # ACT emission count after item i's rowsum = i + NQ + 1 (item 15 has
    # no red of its own -- its total lands in ps[:, 15] via a DVE combine
    # of the quarter partials, so group 2 gates on red 14 = all 19 reds)
    cnt = [i + NQ + 1 for i in range(ni)]
    cmb_gate = [cnt[min(2 * b - 1, ni - 2)] for _, b in GROUPS]

    nc = bass.Bass(num_devices=NCORES)
    mx = nc.dram_tensor("mx", [SHARD, N], F32, kind="ExternalInput")
    eye = nc.dram_tensor("eye", [P, P], F32, kind="ExternalInput")
    out = nc.dram_tensor("out", [SHARD, N], F32, kind="ExternalOutput")
    cc_in = [
        nc.dram_tensor(f"cc_in_{g}", [(b - a) * P], BF16)
        for g, (a, b) in enumerate(GROUPS)
    ]
    cc_out = [
        nc.dram_tensor(
            f"cc_out_{g}", [NCORES, (b - a) * P], BF16, addr_space="Shared"
        )
        for g, (a, b) in enumerate(GROUPS)
    ]

    # tile t, partition p, half h, block b, col v -> shard row t*128 + p,
    # col h*4096 + b*1024 + v
    mx_v = mx.rearrange("(t p) (h w) -> t p h w", p=P, h=H)
    mx_vb = mx.rearrange("(t p) (h b v) -> t p h b v", p=P, h=H, b=B)
    out_v = out.rearrange("(t p) (h b v) -> t p h b v", p=P, h=H, b=B)

    with ExitStack() as ctx:
        slots = [
            ctx.enter_context(nc.sbuf_tensor(f"slot{s}", [P, B, V], F32))
            for s in range(NSLOTS)
        ]
        ded = ctx.enter_context(nc.sbuf_tensor("ded", [P, B, V], F32))
        cache = [
            ctx.enter_context(nc.sbuf_tensor(f"cache{i}", [P, B, V], BF16))
            for i in range(ni - 1)
        ]
        colscale = ctx.enter_context(
            nc.sbuf_tensor("colscale", [P, NCORES, V], BF16)
        )
        eye_sb = ctx.enter_context(nc.sbuf_tensor("eye_sb", [P, P], F32))
        # cols 0-14: items; 16-19: ded quarters; 20-21: combine scratch;
        # after the DVE pre-combine, col 15 holds item 15's total
        ps = ctx.enter_context(nc.sbuf_tensor("ps", [P, ni + NQ + 2], F32))
        rs = ctx.enter_context(nc.sbuf_tensor("rs", [P, T], F32))
        rinv = ctx.enter_context(nc.sbuf_tensor("rinv", [P, T], F32))
        ptc = [
            ctx.enter_context(nc.sbuf_tensor(f"ptc{g}", [b - a, P], BF16))
            for g, (a, b) in enumerate(GROUPS)
        ]
        pt = [
            ctx.enter_context(nc.psum_tensor(f"pt{g}", [b - a, P], F32))
            for g, (a, b) in enumerate(GROUPS)
        ]

        s_in = [
            ctx.enter_context(nc.semaphore(f"s_in{s}")) for s in range(NSLOTS)
        ]
        # ded quarters: q0,q2 ride Pool (FIFO), q1,q3 ride SP (FIFO)
        s_inq = [ctx.enter_context(nc.semaphore(f"s_inq{r}")) for r in range(2)]
        s_sout = [
            [
                ctx.enter_context(nc.semaphore(f"s_sout{x}_{s}"))
                for s in range(NSLOTS)
            ]
            for x in range(NG)
        ]
        s_soutd = ctx.enter_context(nc.semaphore("s_soutd"))  # ded stores
        s_red = ctx.enter_context(nc.semaphore("s_red"))
        s_eye = ctx.enter_context(nc.semaphore("s_eye"))
        s_fold = ctx.enter_context(nc.semaphore("s_fold"))
        s_sqrt = [
            ctx.enter_context(nc.semaphore(f"s_sqrt{g}")) for g in range(NG)
        ]
        s_tp = [ctx.enter_context(nc.semaphore(f"s_tp{g}")) for g in range(NG)]
        s_ptc = [
            ctx.enter_context(nc.semaphore(f"s_ptc{g}")) for g in range(NG)
        ]
        s_ccin = [
            ctx.enter_context(nc.semaphore(f"s_ccin{g}")) for g in range(NG)
        ]
        s_cc = [ctx.enter_context(nc.semaphore(f"s_cc{g}")) for g in range(NG)]
        s_cs = [ctx.enter_context(nc.semaphore(f"s_cs{g}")) for g in range(NG)]
        s_stt = ctx.enter_context(nc.semaphore("s_stt"))
        block = ctx.enter_context(nc.Block())

        def span(x):  # column range of phase x within each 1024-block
            a, b = GROUPS[x]
            return (a * P, b * P)

        def cache_of(i):
            return ded if i == last else cache[i]

        def emit_load(eng, j):
            kind, v = lorder[j]
            if kind == "q":
                src = mx_vb[T - 1, :, 1, v : v + 1, :]
                eng.dma_start(ded[:, v : v + 1, :], src).then_inc(
                    s_inq[v % 2], 16
                )
            else:
                if v >= NSLOTS:
                    eng.wait_ge(s_red, v + 2)  # prev occupant consumed
                eng.dma_start(
                    slots[v % NSLOTS][:, :, :], mx_v[items[v][0], :, items[v][1]]
                ).then_inc(s_in[v % NSLOTS], 16)

        def gather(g, gi):
            # NRT requires straight-line collective ordering on gpsimd
            g.wait_ge(s_ccin[gi], 16)
            g.collective_compute(
                "AllGather",
                mybir.AluOpType.bypass,
                replica_groups=[list(range(NCORES))],
                ins=[cc_in[gi][:]],
                outs=[cc_out[gi][:, :]],
            ).then_inc(s_cc[gi], 1)

        def emit_stores(eng, parity, x, krange=None):
            lo, hi = span(x)
            for k in krange if krange is not None else range(parity, ni, 2):
                t, h = items[k]
                eng.wait_ge(s_stt, ni * x + k + 1)
                sem = s_soutd if k == last else s_sout[x][k % NSLOTS]
                src = (ded if k == last else slots[k % NSLOTS])[:, :, lo:hi]
                eng.dma_start(out_v[t, :, h, :, lo:hi], src).then_inc(sem, 16)

        @block.gpsimd
        def _(g):
            for j in range(0, len(lorder), 2):
                emit_load(g, j)
                if j == 14:  # group-0 rowsums complete around here
                    gather(g, 0)
            gather(g, 1)
            for x in range(NG):
                emit_stores(g, 1, x)

        @block.sync
        def _(sp):
            for j in range(1, len(lorder), 2):
                if j == 5:
                    # eye rides the SP ring early; PE needs it at ~65 us
                    sp.dma_start(eye_sb[:, :], eye[:, :]).then_inc(s_eye, 16)
                emit_load(sp, j)
            for x in range(NG):
                emit_stores(sp, 0, x)
            # all stores landed before halt
            for x in range(NG):
                for s in range(NSLOTS):
                    sp.wait_ge(s_sout[x][s], 16 * 5)
            sp.wait_ge(s_soutd, 16 * NG)

        @block.scalar
        def _(s):
            # pass 1: rowsum partials via Copy-with-accum; the Copy output
            # IS the bf16 cache write (ded chunks copy in place, f32).
            # cc_in writes + colscale broadcasts ride this engine's
            # otherwise-empty DMA queue.
            def red(i):
                s.wait_ge(s_in[i % NSLOTS], 16 * (i // NSLOTS + 1))
                s.activation(
                    cache[i][:, :, :],
                    slots[i % NSLOTS][:, :, :],
                    mybir.ActivationFunctionType.Copy,
                    accum_out=ps[:, i : i + 1],
                ).then_inc(s_red, 1)

            def finish_group(gi):
                # rowsum folds + sqrt stay ON THIS ENGINE: accumulator
                # writebacks land asynchronously, so cross-engine readers
                # gated on s_red can read stale ps.  Self-waits drain them.
                a, b = GROUPS[gi]
                s.wait_ge(s_red, cmb_gate[gi])
                for t in range(a, b):
                    s.activation(
                        ps[:, 2 * t : 2 * t + 2],
                        ps[:, 2 * t : 2 * t + 2],
                        mybir.ActivationFunctionType.Copy,
                        accum_out=rs[:, t : t + 1],
                    ).then_inc(s_fold, 1)
                s.wait_ge(s_fold, sum(bb - aa for aa, bb in GROUPS[: gi + 1]))
                s.sqrt(rs[:, a:b], rs[:, a:b]).then_inc(s_sqrt[gi], 1)
                s.wait_ge(s_ptc[gi], 1)
                s.dma_start(cc_in[gi][:], ptc[gi][:, :]).then_inc(
                    s_ccin[gi], 16
                )

            def bcast(gi):
                lo, hi = span(gi)
                s.wait_ge(s_cc[gi], 1)
                s.dma_start(
                    colscale[:, :, lo:hi],
                    cc_out[gi][:, :].partition_broadcast(P),
                ).then_inc(s_cs[gi], 16)

            for q in range(NQ):
                s.wait_ge(s_inq[q % 2], 16 * (q // 2 + 1))
                s.activation(
                    ded[:, q : q + 1, :],
                    ded[:, q : q + 1, :],
                    mybir.ActivationFunctionType.Copy,
                    accum_out=ps[:, ni + q : ni + q + 1],
                ).then_inc(s_red, 1)
            # fold the quarter partials into ps[:, 15] on THIS engine: the
            # self-wait drains the accum writebacks (they land async; a
            # cross-engine reader gated only on s_red can read stale ps)
            s.wait_ge(s_red, NQ)
            s.activation(
                ps[:, ni : ni + NQ],
                ps[:, ni : ni + NQ],
                mybir.ActivationFunctionType.Copy,
                accum_out=ps[:, last : last + 1],
            )
            for i in range(0, 2 * GROUPS[0][1]):
                red(i)
            finish_group(0)
            for i in range(2 * GROUPS[0][1], ni - 1):
                red(i)
            finish_group(1)
            bcast(0)
            bcast(1)

        @block.tensor
        def _(pe):
            # sqrt(rowsum) [128, g] -> [g, 128] in PSUM (via identity)
            pe.wait_ge(s_eye, 16)
            for gi, (a, b) in enumerate(GROUPS):
                pe.wait_ge(s_sqrt[gi], 1)
                pe.transpose(pt[gi][:, :], rs[:, a:b], eye_sb[:, :]).then_inc(
                    s_tp[gi], 1
                )

        @block.vector
        def _(v):
            assert H == 2

            def chain(gi):
                a, b = GROUPS[gi]
                v.wait_ge(s_sqrt[gi], 1)
                v.reciprocal(rinv[:, a:b], rs[:, a:b])
                v.wait_ge(s_tp[gi], 1)
                with nc.allow_low_precision("bf16 column scale is in-gate"):
                    v.reciprocal(ptc[gi][:, :], pt[gi][:, :]).then_inc(
                        s_ptc[gi], 1
                    )

            def stt(k, x):
                t, h = items[k]
                lo, hi = span(x)
                if k < NSLOTS:
                    # slot k's last pass-1 occupant is item 12+k; don't
                    # overwrite (any span -- data is shared) before it is
                    # consumed.  Matters on slow-started cores, whose
                    # colscale gates fire early in their local timeline.
                    v.wait_ge(s_red, cnt[12 + k])
                if NSLOTS <= k < last:
                    v.wait_ge(s_sout[x][k % NSLOTS], 16 * (k // NSLOTS))
                v.scalar_tensor_tensor(
                    (ded if k == last else slots[k % NSLOTS])[:, :, lo:hi],
                    cache_of(k)[:, :, lo:hi],
                    rinv[:, t : t + 1],
                    colscale[:, B * h : B * (h + 1), lo:hi],
                    op0=mybir.AluOpType.mult,
                    op1=mybir.AluOpType.mult,
                ).then_inc(s_stt, 1)

            # chain(1) must precede any colscale-gated stt: ACT's group-1
            # finish waits on s_cmb[1], and ACT's bcast(0) feeds s_cs[0]
            chain(0)
            chain(1)
            v.wait_ge(s_cs[0], 16)
            for k in range(ni):
                stt(k, 0)
            for x in range(1, NG):
                v.wait_ge(s_cs[x], 16)
                for k in range(ni):
                    stt(k, x)

    return nc


_NC_CACHE = {}
_WARM = False


def _get_nc():
    if "nc" not in _NC_CACHE:
        _NC_CACHE["nc"] = build_kernel()
    return _NC_CACHE["nc"]


def kernel(adj, **run_kwargs):
    adj = np.asarray(adj)
    assert adj.shape == (N, N) and adj.dtype == np.float32
    mx = adj.copy()
    idx = np.arange(N)
    mx[idx, idx] += 1.0
    eye = np.eye(P, dtype=np.float32)

    in_maps = [
        {"mx": mx[c * SHARD : (c + 1) * SHARD], "eye": eye}
        for c in range(NCORES)
    ]
    nc = _get_nc()
    try:
        res = run_bass_kernel_spmd(nc, in_maps, list(range(NCORES)), **run_kwargs)
    except Exception:
        # transient device hiccups (e.g. a wedged core from an earlier
        # process) sometimes clear on a second attempt
        import time

        time.sleep(2.0)
        res = run_bass_kernel_spmd(nc, in_maps, list(range(NCORES)), **run_kwargs)
    out = np.concatenate([res.results[c]["out"] for c in range(NCORES)], axis=0)
    if run_kwargs:
        return out, res
    return out


# revision 33
# speedup vs baseline: 1.0017x; 1.0017x over previous
"""Normalized-adjacency kernel (EstimateAdj.normalize, symmetric=False) for TRN2.

out = mx * r_inv[:, None] * r_inv[None, :]   where mx = adj + I,
r_inv = rowsum(mx) ** -0.5.

Strategy (8 NeuronCores, row-sharded, raw Bass with explicit semaphores):
  - host: add 1.0 to the diagonal (O(n)), split rows into 8 shards
  - device, per core (shard [1024, 8192], half-tiles [128 x 4096], 16 items):
      pass 1: item 15 is loaded FIRST as four single-block quarters into a
              dedicated f32 tile: the first rowsum activation starts ~4 us
              into the kernel (vs ~19 us for a full half-tile), which
              un-jams the whole slot-gated load pipeline.  Items 0..14
              stream through 3 f32 slots, loads alternating Pool/SP rings.
              Each chunk is consumed by ONE scalar-engine activation (Copy
              with accum_out): the accumulator is the rowsum partial and
              the Copy output writes a bf16 replica into a persistent SBUF
              cache (the dedicated tile copies in place, staying f32).
              The 32 MiB shard is cached on-chip, NEVER reloaded from HBM.
      Rowsum folds + sqrt stay ON the scalar engine (accumulator
      writebacks land asynchronously; only self-waits drain them --
      cross-engine readers of accum targets are a race).
      r_inv = 1/sqrt(rowsum), transposed via PE, bf16 via DVE reciprocal.
      The AllGather is SPLIT IN TWO at the tile-4 boundary.  The first
      gather's input is ready at ~55% of the load phase, so its ~12 us
      setup and the ~25-35 us cross-core start skew (the runtime
      staggers core launches) largely complete while loads stream; the
      second rides the warm CC pipeline (~2 us setup) and releases
      before the store stream reaches the columns it gates.
      cc_in writes + colscale broadcasts ride the Scalar engine's
      otherwise-empty DMA queue; collectives trigger on gpsimd (NRT
      requires straight-line gpsimd ordering).
      pass 2: per item and phase x (columns j with (j mod 1024) in
      [512x, 512x+512) are phase-x columns = rows of tile-group x), DVE
      scalar_tensor_tensor over block-strided views:
              slot[:, :, lo:hi] = (cache * r_inv_row) * colscale_x
      then a block-strided store (2 KB descriptors), alternating SP/Pool
      rings.  Stores start the moment the core's own loads drain; the
      collective latency hides under the load/store streams.
  - bf16 rounding of mx and colscale adds ~8e-3 relative error -- inside
    the 2e-2 gate; rowsums/r_inv row scalars stay f32.
  - host: concatenate the 8 output shards

HBM traffic per core: 32 MiB load + 32 MiB store + 2 MiB colscale (vs the
two-pass baseline's 92 MiB) -> DMA-bound floor ~190 us at 360 GB/s.
"""

from contextlib import ExitStack

import numpy as np

import concourse.bass as bass
import concourse.mybir as mybir
from concourse.bass_utils import run_bass_kernel_spmd

N = 8192
NCORES = 8
SHARD = N // NCORES  # 1024
P = 128
T = SHARD // P  # 8 tiles per core
H = 2  # column halves per tile
W = N // H  # 4096
B = 4  # 1024-col blocks per half
V = W // B  # 1024

F32 = mybir.dt.float32
BF16 = mybir.dt.bfloat16
NSLOTS = 3  # f32 streaming slots (pass-1 loads / pass-2 outputs)
GROUPS = [(0, 4), (4, 8)]  # tile groups -> gather phases
NG = len(GROUPS)
NQ = 4  # dedicated-tile load quarters (single blocks)


def build_kernel():
    items = [(t, h) for t in range(T) for h in range(H)]
    ni = len(items)
    last = ni - 1  # item 15: dedicated f32 tile, scaled in place
    # load positions: 4 ded quarters first, then items 0..14
    lorder = [("q", q) for q in range(NQ)] + [("i", i) for i in range(ni - 1)]
    # ACT emission count after item i's rowsum = i + NQ + 1 (item 15 has
    # no red of its own -- its total lands in ps[:, 15] via a DVE combine
    # of the quarter partials, so group 2 gates on red 14 = all 19 reds)
    cnt = [i + NQ + 1 for i in range(ni)]
    cmb_gate = [cnt[min(2 * b - 1, ni - 2)] for _, b in GROUPS]

    nc = bass.Bass(num_devices=NCORES)
    mx = nc.dram_tensor("mx", [SHARD, N], F32, kind="ExternalInput")
    eye = nc.dram_tensor("eye", [P, P], F32, kind="ExternalInput")
    out = nc.dram_tensor("out", [SHARD, N], F32, kind="ExternalOutput")
    cc_in = [
        nc.dram_tensor(f"cc_in_{g}", [(b - a) * P], BF16)
        for g, (a, b) in enumerate(GROUPS)
    ]
    cc_out = [
        nc.dram_tensor(
            f"cc_out_{g}", [NCORES, (b - a) * P], BF16, addr_space="Shared"
        )
        for g, (a, b) in enumerate(GROUPS)
    ]

    # tile t, partition p, half h, block b, col v -> shard row t*128 + p,
    # col h*4096 + b*1024 + v
    mx_v = mx.rearrange("(t p) (h w) -> t p h w", p=P, h=H)
    mx_vb = mx.rearrange("(t p) (h b v) -> t p h b v", p=P, h=H, b=B)
    out_v = out.rearrange("(t p) (h b v) -> t p h b v", p=P, h=H, b=B)

    with ExitStack() as ctx:
        slots = [
            ctx.enter_context(nc.sbuf_tensor(f"slot{s}", [P, B, V], F32))
            for s in range(NSLOTS)
        ]
        ded = ctx.enter_context(nc.sbuf_tensor("ded", [P, B, V], F32))
        cache = [
            ctx.enter_context(nc.sbuf_tensor(f"cache{i}", [P, B, V], BF16))
            for i in range(ni - 1)
        ]
        colscale = ctx.enter_context(
            nc.sbuf_tensor("colscale", [P, NCORES, V], BF16)
        )
        eye_sb = ctx.enter_context(nc.sbuf_tensor("eye_sb", [P, P], F32))
        # cols 0-14: items; 16-19: ded quarters; 20-21: combine scratch;
        # after the DVE pre-combine, col 15 holds item 15's total
        ps = ctx.enter_context(nc.sbuf_tensor("ps", [P, ni + NQ + 2], F32))
        rs = ctx.enter_context(nc.sbuf_tensor("rs", [P, T], F32))
        rinv = ctx.enter_context(nc.sbuf_tensor("rinv", [P, T], F32))
        ptc = [
            ctx.enter_context(nc.sbuf_tensor(f"ptc{g}", [b - a, P], BF16))
            for g, (a, b) in enumerate(GROUPS)
        ]
        pt = [
            ctx.enter_context(nc.psum_tensor(f"pt{g}", [b - a, P], F32))
            for g, (a, b) in enumerate(GROUPS)
        ]

        s_in = [
            ctx.enter_context(nc.semaphore(f"s_in{s}")) for s in range(NSLOTS)
        ]
        # ded quarters: q0,q2 ride Pool (FIFO), q1,q3 ride SP (FIFO)
        s_inq = [ctx.enter_context(nc.semaphore(f"s_inq{r}")) for r in range(2)]
        s_sout = [
            [
                ctx.enter_context(nc.semaphore(f"s_sout{x}_{s}"))
                for s in range(NSLOTS)
            ]
            for x in range(NG)
        ]
        s_soutd = ctx.enter_context(nc.semaphore("s_soutd"))  # ded stores
        s_red = ctx.enter_context(nc.semaphore("s_red"))
        s_eye = ctx.enter_context(nc.semaphore("s_eye"))
        s_fold = ctx.enter_context(nc.semaphore("s_fold"))
        s_sqrt = [
            ctx.enter_context(nc.semaphore(f"s_sqrt{g}")) for g in range(NG)
        ]
        s_tp = [ctx.enter_context(nc.semaphore(f"s_tp{g}")) for g in range(NG)]
        s_ptc = [
            ctx.enter_context(nc.semaphore(f"s_ptc{g}")) for g in range(NG)
        ]
        s_ccin = [
            ctx.enter_context(nc.semaphore(f"s_ccin{g}")) for g in range(NG)
        ]
        s_cc = [ctx.enter_context(nc.semaphore(f"s_cc{g}")) for g in range(NG)]
        s_cs = [ctx.enter_context(nc.semaphore(f"s_cs{g}")) for g in range(NG)]
        s_stt = ctx.enter_context(nc.semaphore("s_stt"))
        block = ctx.enter_context(nc.Block())

        def span(x):  # column range of phase x within each 1024-block
            a, b = GROUPS[x]
            return (a * P, b * P)

        def cache_of(i):
            return ded if i == last else cache[i]

        def emit_load(eng, j):
            kind, v = lorder[j]
            if kind == "q":
                src = mx_vb[T - 1, :, 1, v : v + 1, :]
                eng.dma_start(ded[:, v : v + 1, :], src).then_inc(
                    s_inq[v % 2], 16
                )
            else:
                if v >= NSLOTS:
                    eng.wait_ge(s_red, v + 2)  # prev occupant consumed
                eng.dma_start(
                    slots[v % NSLOTS][:, :, :], mx_v[items[v][0], :, items[v][1]]
                ).then_inc(s_in[v % NSLOTS], 16)

        def gather(g, gi):
            # NRT requires straight-line collective ordering on gpsimd
            g.wait_ge(s_ccin[gi], 16)
            g.collective_compute(
                "AllGather",
                mybir.AluOpType.bypass,
                replica_groups=[list(range(NCORES))],
                ins=[cc_in[gi][:]],
                outs=[cc_out[gi][:, :]],
            ).then_inc(s_cc[gi], 1)

        def emit_stores(eng, parity, x, krange=None):
            lo, hi = span(x)
            for k in krange if krange is not None else range(parity, ni, 2):
                t, h = items[k]
                eng.wait_ge(s_stt, ni * x + k + 1)
                sem = s_soutd if k == last else s_sout[x][k % NSLOTS]
                src = (ded if k == last else slots[k % NSLOTS])[:, :, lo:hi]
                eng.dma_start(out_v[t, :, h, :, lo:hi], src).then_inc(sem, 16)

        @block.gpsimd
        def _(g):
            for j in range(0, len(lorder), 2):
                emit_load(g, j)
                if j == 14:  # group-0 rowsums complete around here
                    gather(g, 0)
            gather(g, 1)
            for x in range(NG):
                emit_stores(g, 1, x)

        @block.sync
        def _(sp):
            for j in range(1, len(lorder), 2):
                if j == 5:
                    # eye rides the SP ring early; PE needs it at ~65 us
                    sp.dma_start(eye_sb[:, :], eye[:, :]).then_inc(s_eye, 16)
                emit_load(sp, j)
            for x in range(NG):
                emit_stores(sp, 0, x)
            # all stores landed before halt
            for x in range(NG):
                for s in range(NSLOTS):
                    sp.wait_ge(s_sout[x][s], 16 * 5)
            sp.wait_ge(s_soutd, 16 * NG)

        @block.scalar
        def _(s):
            # pass 1: rowsum partials via Copy-with-accum; the Copy output
            # IS the bf16 cache write (ded chunks copy in place, f32).
            # cc_in writes + colscale broadcasts ride this engine's
            # otherwise-empty DMA queue.
            def red(i):
                s.wait_ge(s_in[i % NSLOTS], 16 * (i // NSLOTS + 1))
                s.activation(
                    cache[i][:, :, :],
                    slots[i % NSLOTS][:, :, :],
                    mybir.ActivationFunctionType.Copy,
                    accum_out=ps[:, i : i + 1],
                ).then_inc(s_red, 1)

            def finish_sqrt(gi):
                # rowsum folds + sqrt stay ON THIS ENGINE: accumulator
                # writebacks land asynchronously, so cross-engine readers
                # gated on s_red can read stale ps.  Self-waits drain them.
                a, b = GROUPS[gi]
                s.wait_ge(s_red, cmb_gate[gi])
                for t in range(a, b):
                    s.activation(
                        ps[:, 2 * t : 2 * t + 2],
                        ps[:, 2 * t : 2 * t + 2],
                        mybir.ActivationFunctionType.Copy,
                        accum_out=rs[:, t : t + 1],
                    ).then_inc(s_fold, 1)
                s.wait_ge(s_fold, sum(bb - aa for aa, bb in GROUPS[: gi + 1]))
                s.sqrt(rs[:, a:b], rs[:, a:b]).then_inc(s_sqrt[gi], 1)

            def finish_ccin(gi):
                # emitted a couple of reds later so the PE->DVE round trip
                # (s_ptc) hides instead of stalling the red stream, which
                # gates slot-reuse loads
                s.wait_ge(s_ptc[gi], 1)
                s.dma_start(cc_in[gi][:], ptc[gi][:, :]).then_inc(
                    s_ccin[gi], 16
                )

            def bcast(gi):
                lo, hi = span(gi)
                s.wait_ge(s_cc[gi], 1)
                s.dma_start(
                    colscale[:, :, lo:hi],
                    cc_out[gi][:, :].partition_broadcast(P),
                ).then_inc(s_cs[gi], 16)

            for q in range(NQ):
                s.wait_ge(s_inq[q % 2], 16 * (q // 2 + 1))
                s.activation(
                    ded[:, q : q + 1, :],
                    ded[:, q : q + 1, :],
                    mybir.ActivationFunctionType.Copy,
                    accum_out=ps[:, ni + q : ni + q + 1],
                ).then_inc(s_red, 1)
            # fold the quarter partials into ps[:, 15] on THIS engine: the
            # self-wait drains the accum writebacks (they land async; a
            # cross-engine reader gated only on s_red can read stale ps)
            s.wait_ge(s_red, NQ)
            s.activation(
                ps[:, ni : ni + NQ],
                ps[:, ni : ni + NQ],
                mybir.ActivationFunctionType.Copy,
                accum_out=ps[:, last : last + 1],
            )
            for i in range(0, 2 * GROUPS[0][1]):
                red(i)
            finish_sqrt(0)
            red(2 * GROUPS[0][1])
            red(2 * GROUPS[0][1] + 1)
            finish_ccin(0)
            for i in range(2 * GROUPS[0][1] + 2, ni - 1):
                red(i)
            finish_sqrt(1)
            finish_ccin(1)
            bcast(0)
            bcast(1)

        @block.tensor
        def _(pe):
            # sqrt(rowsum) [128, g] -> [g, 128] in PSUM (via identity)
            pe.wait_ge(s_eye, 16)
            for gi, (a, b) in enumerate(GROUPS):
                pe.wait_ge(s_sqrt[gi], 1)
                pe.transpose(pt[gi][:, :], rs[:, a:b], eye_sb[:, :]).then_inc(
                    s_tp[gi], 1
                )

        @block.vector
        def _(v):
            assert H == 2

            def chain(gi):
                a, b = GROUPS[gi]
                v.wait_ge(s_sqrt[gi], 1)
                v.reciprocal(rinv[:, a:b], rs[:, a:b])
                v.wait_ge(s_tp[gi], 1)
                with nc.allow_low_precision("bf16 column scale is in-gate"):
                    v.reciprocal(ptc[gi][:, :], pt[gi][:, :]).then_inc(
                        s_ptc[gi], 1
                    )

            def stt(k, x):
                t, h = items[k]
                lo, hi = span(x)
                if k < NSLOTS:
                    # slot k's last pass-1 occupant is item 12+k; don't
                    # overwrite (any span -- data is shared) before it is
                    # consumed.  Matters on slow-started cores, whose
                    # colscale gates fire early in their local timeline.
                    v.wait_ge(s_red, cnt[12 + k])
                if NSLOTS <= k < last:
                    v.wait_ge(s_sout[x][k % NSLOTS], 16 * (k // NSLOTS))
                v.scalar_tensor_tensor(
                    (ded if k == last else slots[k % NSLOTS])[:, :, lo:hi],
                    cache_of(k)[:, :, lo:hi],
                    rinv[:, t : t + 1],
                    colscale[:, B * h : B * (h + 1), lo:hi],
                    op0=mybir.AluOpType.mult,
                    op1=mybir.AluOpType.mult,
                ).then_inc(s_stt, 1)

            # chain(1) must precede any colscale-gated stt: ACT's group-1
            # finish waits on s_cmb[1], and ACT's bcast(0) feeds s_cs[0]
            chain(0)
            chain(1)
            v.wait_ge(s_cs[0], 16)
            for k in range(ni):
                stt(k, 0)
            for x in range(1, NG):
                v.wait_ge(s_cs[x], 16)
                for k in range(ni):
                    stt(k, x)

    return nc


_NC_CACHE = {}
_WARM = False


def _get_nc():
    if "nc" not in _NC_CACHE:
        _NC_CACHE["nc"] = build_kernel()
    return _NC_CACHE["nc"]


def kernel(adj, **run_kwargs):
    adj = np.asarray(adj)
    assert adj.shape == (N, N) and adj.dtype == np.float32
    mx = adj.copy()
    idx = np.arange(N)
    mx[idx, idx] += 1.0
    eye = np.eye(P, dtype=np.float32)

    in_maps = [
        {"mx": mx[c * SHARD : (c + 1) * SHARD], "eye": eye}
        for c in range(NCORES)
    ]
    nc = _get_nc()
    try:
        res = run_bass_kernel_spmd(nc, in_maps, list(range(NCORES)), **run_kwargs)
    except Exception:
        # transient device hiccups (e.g. a wedged core from an earlier
        # process) sometimes clear on a second attempt
        import time

        time.sleep(2.0)
        res = run_bass_kernel_spmd(nc, in_maps, list(range(NCORES)), **run_kwargs)
    out = np.concatenate([res.results[c]["out"] for c in range(NCORES)], axis=0)
    if run_kwargs:
        return out, res
    return out
